# revision 13
# baseline (speedup 1.0000x reference)
"""ChebGCN (K=2, 3 layers) Trainium2 Bass kernel.

Strategy (1D graph/data parallel, dst-sharded):
  - Host: convert edge list -> dense adjacency COUNT strips AdjT[src, dst_local]
    per core (fp8 e4m3: small integer counts are exact), pad N 10000->10240,
    shard dst rows 1280/core. Pure format conversion; all FP math on device.
  - Device (SPMD on 8 cores):
      dis = sqrt(min(deg,1)/max(deg,1)) computed on device from integer counts.
      Per layer, the Chebyshev term  L_hat x = -D A D x  is a dense tensor-
      engine matmul  T^T = (dis .* X)^T @ AdjT  (fp16 x fp8), scaled by
      -dis_dst at PSUM evacuation. Dense W0/W1 matmuls run feature-major;
      layer outputs are PE-transposed to node-major only for the gather.
  - The adjacency matmul iterates dst-column chunks OUTER (512/512/256), so
    each chunk's result is ready early; its dense part + AllGather piece
    overlap the next chunk's accumulation. Source chunks are consumed in a
    host-side permutation (SIGMA) matching the gathered piece layout.
  - Layer 3 folds W13 before the gather (Y3 = (dis.*h2) @ W13), halving the
    final dense-adjacency matmul width.

kernel(**inputs) takes the FULL unsharded inputs and returns the FULL output.
"""

import os
import sys

sys.path.insert(0, "/opt/trn_rl_repo")

import numpy as np
import ml_dtypes

N = 10000
NP = 10240           # padded node count
NCORES = 8
MLOC = NP // NCORES  # 1280 dst rows per core
P = 128
KT = NP // P         # 80 source chunks of 128
TPC = MLOC // P      # 10 dst tiles per core
D_IN, D_HID, D_OUT = 128, 256, 128
# dst column chunks == gather pieces (psum-bank sized)
CH = [(0, 512), (512, 512), (1024, 256)]
PIECE_TILES = [range(0, 4), range(4, 8), range(8, 10)]
GPP = [32, 32, 16]        # global chunks per piece (8 cores x tiles)
PSTART = [0, 32, 64]      # first sigma position of each piece

# sigma position -> global chunk: piece-major, then (core, tile)
SIGMA = [c * TPC + t for ts in PIECE_TILES for c in range(NCORES) for t in ts]


def _pos(j):
    """sigma position -> (piece index, slot within piece)"""
    for pi in range(3):
        if j < PSTART[pi] + GPP[pi]:
            return pi, j - PSTART[pi]
    raise ValueError(j)


_CACHE = {}
LAST_RESULTS = None  # BassKernelResults of the most recent run (for profiling)


def _build_nc():
    from contextlib import ExitStack

    import concourse.bass as bass
    import concourse.tile as tile
    from concourse import bacc, mybir
    from concourse.masks import make_identity

    f32 = mybir.dt.float32
    f16 = mybir.dt.float16
    f8 = mybir.dt.float8e4
    AF = mybir.ActivationFunctionType
    MUL = mybir.AluOpType.mult

    nc = bacc.Bacc(trn_type="TRN2", num_devices=NCORES)

    adjT_d = nc.dram_tensor("adjT", [KT, P, MLOC], f8, kind="ExternalInput")
    x_nm_d = nc.dram_tensor("x_nm", [KT, P, D_IN], f32, kind="ExternalInput")
    xT_d = nc.dram_tensor("xT_loc", [P, MLOC], f32, kind="ExternalInput")
    degc_d = nc.dram_tensor("deg_cols", [P, KT], f32, kind="ExternalInput")
    degr_d = nc.dram_tensor("deg_row", [1, MLOC], f32, kind="ExternalInput")
    w01_d = nc.dram_tensor("w01", [P, D_HID], f32, kind="ExternalInput")
    w11_d = nc.dram_tensor("w11", [P, D_HID], f32, kind="ExternalInput")
    w02_d = nc.dram_tensor("w02", [2, P, D_HID], f32, kind="ExternalInput")
    w12_d = nc.dram_tensor("w12", [2, P, D_HID], f32, kind="ExternalInput")
    w03_d = nc.dram_tensor("w03", [2, P, D_OUT], f32, kind="ExternalInput")
    w13_d = nc.dram_tensor("w13", [2, P, D_OUT], f32, kind="ExternalInput")
    b1_d = nc.dram_tensor("b1r", [1, D_HID], f32, kind="ExternalInput")
    b2_d = nc.dram_tensor("b2r", [1, D_HID], f32, kind="ExternalInput")
    b3_d = nc.dram_tensor("b3r", [1, D_OUT], f32, kind="ExternalInput")
    out_d = nc.dram_tensor("outT", [P, MLOC], f32, kind="ExternalOutput")

    with tile.TileContext(nc) as tc, ExitStack() as ctx:
        const = ctx.enter_context(tc.tile_pool(name="const", bufs=1))
        stage = ctx.enter_context(tc.tile_pool(name="stage", bufs=1))
        io = ctx.enter_context(tc.tile_pool(name="io", bufs=4))
        xsl = ctx.enter_context(tc.tile_pool(name="xsl", bufs=2))
        adjp = ctx.enter_context(tc.tile_pool(name="adjp", bufs=6))
        feat = ctx.enter_context(tc.tile_pool(name="feat", bufs=1))
        pbig = ctx.enter_context(tc.tile_pool(name="pbig", bufs=3, space="PSUM"))
        pout = ctx.enter_context(tc.tile_pool(name="pout", bufs=3, space="PSUM"))
        ptr = ctx.enter_context(tc.tile_pool(name="ptr", bufs=2, space="PSUM"))
        dram = ctx.enter_context(tc.tile_pool(name="dram", bufs=1, space="DRAM"))

        adjT = adjT_d[:]
        x_nm = x_nm_d[:]

        # ---------- degree -> dis on device (emitted first: feeds L1) -------
        def make_dis(name, dtensor, shape):
            # dis = sqrt(min(deg,1) * 1/max(deg,1)); all-DVE chain, one ACT sqrt
            dg = stage.tile(shape, f32, name=f"{name}_dg")
            nc.sync.dma_start(dg[:], dtensor[:])
            tmp = stage.tile(shape, f32, name=f"{name}_tmp")
            nc.vector.tensor_scalar_max(tmp[:], dg[:], 1.0)
            nc.vector.reciprocal(tmp[:], tmp[:])
            msk = stage.tile(shape, f32, name=f"{name}_msk")
            nc.vector.tensor_scalar_min(msk[:], dg[:], 1.0)
            nc.vector.tensor_tensor(tmp[:], tmp[:], msk[:], MUL)
            dis = const.tile(shape, f32, name=name)
            nc.scalar.activation(dis[:], tmp[:], AF.Sqrt)
            return dis

        dis_cols = make_dis("dis_cols", degc_d, [P, KT])  # dis over src (sigma order)
        dis_row = make_dis("dis_row", degr_d, [1, MLOC])  # dis over local dst

        # broadcast rows: ndis_bc[q, j] = -dis_row[j]; pdis_bc = +dis_row
        ones1f = const.tile([1, P], f32)
        nc.gpsimd.memset(ones1f[:], 1.0)
        ndis_row = const.tile([1, MLOC], f32)
        nc.vector.tensor_scalar_mul(ndis_row[:], dis_row[:], -1.0)
        ndis_bc = const.tile([P, MLOC], f32)
        pdis_bc = const.tile([P, MLOC], f16)
        for c0, cw in CH:
            pb = pout.tile([P, 512], f32, name="pb_bc", tag="po")
            nc.tensor.matmul(pb[:, :cw], ones1f[:], ndis_row[:, c0 : c0 + cw])
            nc.vector.tensor_copy(ndis_bc[:, c0 : c0 + cw], pb[:, :cw])
            nc.vector.tensor_scalar_mul(pdis_bc[:, c0 : c0 + cw], pb[:, :cw], -1.0)

        # ---------- x: scaled node-major pieces (slab DMA + DVE scale) ------
        xs1p = [feat.tile([P, GPP[p], D_IN], f16, name=f"xs1_{p}") for p in range(3)]
        SLAB = 16
        for s0 in range(0, KT, SLAB):
            xslab = xsl.tile([P, SLAB, D_IN], f32, name="xslab")
            nc.sync.dma_start(xslab[:], x_nm[s0 : s0 + SLAB].rearrange("k q d -> q k d"))
            for k in range(SLAB):
                j = s0 + k
                p, r = _pos(j)
                nc.vector.tensor_scalar(
                    out=xs1p[p][:, r, :],
                    in0=xslab[:, k, :],
                    scalar1=dis_cols[:, j : j + 1],
                    scalar2=None,
                    op0=MUL,
                )

        # ---------- constants / weights ----------
        id16 = const.tile([P, P], f16)
        make_identity(nc, id16)
        id32 = const.tile([P, P], f32)
        make_identity(nc, id32)
        onesrow = const.tile([1, MLOC], f16)
        nc.gpsimd.memset(onesrow[:], 1.0)

        def load_cast(name, dtensor, shape):
            wf = stage.tile(shape, f32, name=f"{name}_f")
            nc.sync.dma_start(wf[:], dtensor[:])
            wh = const.tile(shape, f16, name=name)
            nc.vector.tensor_copy(wh[:], wf[:])
            return wh

        w01h = load_cast("w01h", w01_d, [P, D_HID])
        w11h = load_cast("w11h", w11_d, [P, D_HID])
        w02h = load_cast("w02h", w02_d[:].rearrange("b p w -> p b w"), [P, 2, D_HID])
        w12h = load_cast("w12h", w12_d[:].rearrange("b p w -> p b w"), [P, 2, D_HID])
        w03h = load_cast("w03h", w03_d[:].rearrange("b p w -> p b w"), [P, 2, D_OUT])
        w13h = load_cast("w13h", w13_d[:].rearrange("b p w -> p b w"), [P, 2, D_OUT])
        b1h = load_cast("b1h", b1_d, [1, D_HID])
        b2h = load_cast("b2h", b2_d, [1, D_HID])
        b3h = load_cast("b3h", b3_d, [1, D_OUT])

        xT16 = const.tile([P, MLOC], f16)
        xTf = stage.tile([P, MLOC], f32, name="xTf")
        nc.sync.dma_start(xTf[:], xT_d[:])
        nc.vector.tensor_copy(xT16[:], xTf[:])

        # ---------- chunk-outer adjacency matmul ----------
        def big_chunk(ci, blocks, ts_dst):
            """Accumulate T^T[:, chunk ci] over all 80 source chunks.
            blocks: list of (feat_block_fn, ts_out_ap) pairs, one per 128-row
            feature block (d=256 runs both against one adjacency stream)."""
            c0, cw = CH[ci]
            accs = [pbig.tile([P, 512], f32, name=f"acc{b}", tag="acc") for b in range(len(blocks))]
            for j in range(KT):
                at = adjp.tile([P, 512], f8, name="at")
                nc.sync.dma_start(at[:, :cw], adjT[j, :, c0 : c0 + cw])
                for b, (fb, _) in enumerate(blocks):
                    nc.tensor.matmul(
                        accs[b][:, :cw], fb(j), at[:, :cw],
                        start=(j == 0), stop=(j == KT - 1),
                    )
            for b, (_, out_ap) in enumerate(blocks):
                nc.vector.tensor_tensor(out_ap, accs[b][:, :cw], ndis_bc[:, c0 : c0 + cw], MUL)

        # transpose feature-major blocks to node-major tiles + bounce + gather
        def emit_gather(ci, srcT, blocks, width, bounce, bounce_v, gathered, gout):
            c0, cw = CH[ci]
            for t in PIECE_TILES[ci]:
                hst = io.tile([P, width], f16, name="hst", tag="hst")
                for b in range(blocks):
                    ptt = ptr.tile([P, P], f16, name="ptt")
                    nc.tensor.transpose(ptt[:], srcT(b, t), id16[:])
                    nc.vector.tensor_copy(hst[:, b * P : (b + 1) * P], ptt[:])
                nc.sync.dma_start(bounce_v[:, t, :], hst[:])
            nc.gpsimd.collective_compute(
                "AllGather",
                mybir.AluOpType.bypass,
                replica_groups=[list(range(NCORES))],
                ins=[bounce[c0 : c0 + cw, :]],
                outs=[gathered[:]],
            )
            nc.sync.dma_start(gout[:], gathered[:].rearrange("(g q) d -> q g d", q=P))

        # ================= Layer 1 =================
        t1s = feat.tile([P, MLOC], f16)
        h1T = feat.tile([P, 2, MLOC], f16)
        h1sT = feat.tile([P, 2, MLOC], f16)
        h1s_bounce = dram.tile([MLOC, D_HID], f16, name="h1s_bounce")
        h1s_bounce_v = h1s_bounce[:].rearrange("(t q) d -> q t d", q=P)
        y2fp = [feat.tile([P, GPP[p], D_HID], f16, name=f"y2f_{p}") for p in range(3)]
        h1s_gath = [
            dram.tile([GPP[p] * P, D_HID], f16, name=f"h1s_gath{p}", addr_space="Shared")
            for p in range(3)
        ]

        for ci, (c0, cw) in enumerate(CH):
            cs = slice(c0, c0 + cw)
            big_chunk(
                ci,
                [(lambda j: xs1p[_pos(j)[0]][:, _pos(j)[1], :], t1s[:, cs])],
                None,
            )
            for bo in range(2):
                bs = slice(bo * P, (bo + 1) * P)
                po = pout.tile([P, 512], f32, name="po")
                nc.tensor.matmul(po[:, :cw], w01h[:, bs], xT16[:, cs], start=True, stop=False)
                nc.tensor.matmul(po[:, :cw], w11h[:, bs], t1s[:, cs], start=False, stop=False)
                nc.tensor.matmul(po[:, :cw], b1h[:, bs], onesrow[:, cs], start=False, stop=True)
                nc.vector.tensor_relu(h1T[:, bo, cs], po[:, :cw])
            for b in range(2):
                nc.vector.tensor_tensor(h1sT[:, b, cs], h1T[:, b, cs], pdis_bc[:, cs], MUL)
            emit_gather(
                ci,
                lambda b, t: h1sT[:, b, t * P : (t + 1) * P],
                2,
                D_HID,
                h1s_bounce,
                h1s_bounce_v,
                h1s_gath[ci],
                y2fp[ci][:],
            )

        # ================= Layer 2 =================
        t2s = feat.tile([P, 2, MLOC], f16)
        h2T = feat.tile([P, 2, MLOC], f16)
        h2sT = feat.tile([P, 2, MLOC], f16)
        y3T = feat.tile([P, MLOC], f16)
        y3_bounce = dram.tile([MLOC, D_OUT], f16, name="y3_bounce")
        y3_bounce_v = y3_bounce[:].rearrange("(t q) d -> q t d", q=P)
        y3fp = [feat.tile([P, GPP[p], D_OUT], f16, name=f"y3f_{p}") for p in range(3)]
        y3_gath = [
            dram.tile([GPP[p] * P, D_OUT], f16, name=f"y3_gath{p}", addr_space="Shared")
            for p in range(3)
        ]

        for ci, (c0, cw) in enumerate(CH):
            cs = slice(c0, c0 + cw)
            big_chunk(
                ci,
                [
                    (lambda j, b=b: y2fp[_pos(j)[0]][:, _pos(j)[1], b * P : (b + 1) * P],
                     t2s[:, b, cs])
                    for b in range(2)
                ],
                None,
            )
            for bo in range(2):
                bs = slice(bo * P, (bo + 1) * P)
                po = pout.tile([P, 512], f32, name="po")
                nc.tensor.matmul(po[:, :cw], w02h[:, 0, bs], h1T[:, 0, cs], start=True, stop=False)
                nc.tensor.matmul(po[:, :cw], w02h[:, 1, bs], h1T[:, 1, cs], start=False, stop=False)
                nc.tensor.matmul(po[:, :cw], w12h[:, 0, bs], t2s[:, 0, cs], start=False, stop=False)
                nc.tensor.matmul(po[:, :cw], w12h[:, 1, bs], t2s[:, 1, cs], start=False, stop=False)
                nc.tensor.matmul(po[:, :cw], b2h[:, bs], onesrow[:, cs], start=False, stop=True)
                nc.vector.tensor_relu(h2T[:, bo, cs], po[:, :cw])
            for b in range(2):
                nc.vector.tensor_tensor(h2sT[:, b, cs], h2T[:, b, cs], pdis_bc[:, cs], MUL)
            # Y3 = (dis .* h2) @ W13, feature-major
            py = pout.tile([P, 512], f32, name="po")
            nc.tensor.matmul(py[:, :cw], w13h[:, 0, :], h2sT[:, 0, cs], start=True, stop=False)
            nc.tensor.matmul(py[:, :cw], w13h[:, 1, :], h2sT[:, 1, cs], start=False, stop=True)
            nc.vector.tensor_copy(y3T[:, cs], py[:, :cw])
            emit_gather(
                ci,
                lambda b, t: y3T[:, t * P : (t + 1) * P],
                1,
                D_OUT,
                y3_bounce,
                y3_bounce_v,
                y3_gath[ci],
                y3fp[ci][:],
            )

        # ================= Layer 3 =================
        t3s = feat.tile([P, MLOC], f32)
        outT = feat.tile([P, MLOC], f32)
        for ci, (c0, cw) in enumerate(CH):
            cs = slice(c0, c0 + cw)
            big_chunk(
                ci,
                [(lambda j: y3fp[_pos(j)[0]][:, _pos(j)[1], :], t3s[:, cs])],
                None,
            )
            po = pout.tile([P, 512], f32, name="po")
            nc.tensor.matmul(po[:, :cw], w03h[:, 0, :], h2T[:, 0, cs], start=True, stop=False)
            nc.tensor.matmul(po[:, :cw], w03h[:, 1, :], h2T[:, 1, cs], start=False, stop=False)
            nc.tensor.matmul(po[:, :cw], b3h[:], onesrow[:, cs], start=False, stop=False)
            # += T3s (identity-matmul add of the scaled Chebyshev term)
            nc.tensor.matmul(po[:, :cw], id32[:], t3s[:, cs], start=False, stop=True)
            nc.vector.tensor_copy(outT[:, cs], po[:, :cw])
            nc.sync.dma_start(out_d[:, cs], outT[:, cs])

    nc.compile()
    return nc


def _prep_inputs(x, edge_index, W01, W11, b1, W02, W12, b2, W03, W13, b3):
    f8 = ml_dtypes.float8_e4m3
    x = np.asarray(x, np.float32)
    ei = np.asarray(edge_index)
    src = ei[0].astype(np.int64)
    dst = ei[1].astype(np.int64)

    deg = np.bincount(src, minlength=NP).astype(np.float32)  # out-degree counts
    x_pad = np.zeros((NP, D_IN), np.float32)
    x_pad[:N] = x

    sig = np.asarray(SIGMA)
    x_nm = np.ascontiguousarray(x_pad.reshape(KT, P, D_IN)[sig])
    deg_cols = np.ascontiguousarray(deg.reshape(KT, P)[sig].T)

    common = {
        "x_nm": x_nm,
        "deg_cols": deg_cols,
        "w01": np.ascontiguousarray(np.asarray(W01, np.float32)),
        "w11": np.ascontiguousarray(np.asarray(W11, np.float32)),
        "w02": np.ascontiguousarray(np.asarray(W02, np.float32).reshape(2, P, D_HID)),
        "w12": np.ascontiguousarray(np.asarray(W12, np.float32).reshape(2, P, D_HID)),
        "w03": np.ascontiguousarray(np.asarray(W03, np.float32).reshape(2, P, D_OUT)),
        "w13": np.ascontiguousarray(np.asarray(W13, np.float32).reshape(2, P, D_OUT)),
        "b1r": np.asarray(b1, np.float32).reshape(1, D_HID),
        "b2r": np.asarray(b2, np.float32).reshape(1, D_HID),
        "b3r": np.asarray(b3, np.float32).reshape(1, D_OUT),
    }

    in_maps = []
    for c in range(NCORES):
        lo, hi = c * MLOC, (c + 1) * MLOC
        sel = (dst >= lo) & (dst < hi)
        idx = src[sel] * MLOC + (dst[sel] - lo)
        cnt = np.bincount(idx, minlength=NP * MLOC).astype(np.float32)
        adjT = np.ascontiguousarray(cnt.reshape(KT, P, MLOC)[sig]).astype(f8)
        m = dict(common)
        m["adjT"] = adjT
        m["xT_loc"] = np.ascontiguousarray(x_pad[lo:hi].T)
        m["deg_row"] = np.ascontiguousarray(deg[lo:hi].reshape(1, MLOC))
        in_maps.append(m)
    return in_maps


def kernel(x, edge_index, edge_type, W01, W11, b1, W02, W12, b2, W03, W13, b3):
    global LAST_RESULTS
    from concourse.bass_utils import run_bass_kernel_spmd

    if "nc" not in _CACHE:
        _CACHE["nc"] = _build_nc()
    nc = _CACHE["nc"]

    in_maps = _prep_inputs(x, edge_index, W01, W11, b1, W02, W12, b2, W03, W13, b3)
    res = run_bass_kernel_spmd(
        nc,
        in_maps,
        list(range(NCORES)),
        trace=bool(os.environ.get("BASS_TRACE")),
    )
    LAST_RESULTS = res
    shards = [res.results[c]["outT"].astype(np.float32).T for c in range(NCORES)]
    out = np.concatenate(shards, axis=0)[:N]
    return np.ascontiguousarray(out)


if __name__ == "__main__":
    _build_nc()
    print("build ok")


# revision 16
# speedup vs baseline: 1.6904x; 1.6904x over previous
"""ChebGCN (K=2, 3 layers) Trainium2 Bass kernel.

Strategy (1D graph/data parallel, dst-sharded):
  - Host: convert edge list -> dense adjacency COUNT strips AdjT[src, dst_local]
    per core (fp8 e4m3: small integer counts are exact), pad N 10000->10240,
    shard dst rows 1280/core. Pure format conversion; all FP math on device.
  - Device (SPMD on 8 cores):
      dis = sqrt(min(deg,1)/max(deg,1)) computed on device from integer counts.
      Per layer, the Chebyshev term  L_hat x = -D A D x  is a dense tensor-
      engine matmul  T^T = (dis .* X)^T @ AdjT  (fp16 x fp8), scaled by
      -dis_dst at PSUM evacuation. Dense W0/W1 matmuls run feature-major;
      layer outputs are PE-transposed to node-major only for the gather.
  - The adjacency matmul iterates dst-column chunks OUTER (512/512/256), so
    each chunk's result is ready early; its dense part + AllGather piece
    overlap the next chunk's accumulation. Source chunks are consumed in a
    host-side permutation (SIGMA) matching the gathered piece layout.
  - Layer 3 folds W13 before the gather (Y3 = (dis.*h2) @ W13), halving the
    final dense-adjacency matmul width.

kernel(**inputs) takes the FULL unsharded inputs and returns the FULL output.
"""

import os
import sys

sys.path.insert(0, "/opt/trn_rl_repo")

import numpy as np
import ml_dtypes

N = 10000
NP = 10240           # padded node count
NCORES = 8
MLOC = NP // NCORES  # 1280 dst rows per core
P = 128
KT = NP // P         # 80 source chunks of 128
TPC = MLOC // P      # 10 dst tiles per core
D_IN, D_HID, D_OUT = 128, 256, 128
# dst column chunks == gather pieces (psum-bank sized)
CH = [(0, 512), (512, 512), (1024, 256)]
PIECE_TILES = [range(0, 4), range(4, 8), range(8, 10)]
GPP = [32, 32, 16]        # global chunks per piece (8 cores x tiles)
PSTART = [0, 32, 64]      # first sigma position of each piece

# sigma position -> global chunk: piece-major, then (core, tile)
SIGMA = [c * TPC + t for ts in PIECE_TILES for c in range(NCORES) for t in ts]


def _pos(j):
    """sigma position -> (piece index, slot within piece)"""
    for pi in range(3):
        if j < PSTART[pi] + GPP[pi]:
            return pi, j - PSTART[pi]
    raise ValueError(j)


_CACHE = {}
LAST_RESULTS = None  # BassKernelResults of the most recent run (for profiling)


def _build_nc():
    from contextlib import ExitStack

    import concourse.bass as bass
    import concourse.tile as tile
    from concourse import bacc, mybir
    from concourse.masks import make_identity

    f32 = mybir.dt.float32
    f16 = mybir.dt.float16
    f8 = mybir.dt.float8e4
    AF = mybir.ActivationFunctionType
    MUL = mybir.AluOpType.mult

    nc = bacc.Bacc(trn_type="TRN2", num_devices=NCORES)

    adjT_d = nc.dram_tensor("adjT", [KT, P, MLOC], f8, kind="ExternalInput")
    x_nm_d = nc.dram_tensor("x_nm", [KT, P, D_IN], f32, kind="ExternalInput")
    xT_d = nc.dram_tensor("xT_loc", [P, MLOC], f32, kind="ExternalInput")
    degc_d = nc.dram_tensor("deg_cols", [P, KT], f32, kind="ExternalInput")
    degr_d = nc.dram_tensor("deg_row", [1, MLOC], f32, kind="ExternalInput")
    w01_d = nc.dram_tensor("w01", [P, D_HID], f32, kind="ExternalInput")
    w11_d = nc.dram_tensor("w11", [P, D_HID], f32, kind="ExternalInput")
    w02_d = nc.dram_tensor("w02", [2, P, D_HID], f32, kind="ExternalInput")
    w12_d = nc.dram_tensor("w12", [2, P, D_HID], f32, kind="ExternalInput")
    w03_d = nc.dram_tensor("w03", [2, P, D_OUT], f32, kind="ExternalInput")
    w13_d = nc.dram_tensor("w13", [2, P, D_OUT], f32, kind="ExternalInput")
    b1_d = nc.dram_tensor("b1r", [1, D_HID], f32, kind="ExternalInput")
    b2_d = nc.dram_tensor("b2r", [1, D_HID], f32, kind="ExternalInput")
    b3_d = nc.dram_tensor("b3r", [1, D_OUT], f32, kind="ExternalInput")
    out_d = nc.dram_tensor("outT", [P, MLOC], f32, kind="ExternalOutput")

    with tile.TileContext(nc) as tc, ExitStack() as ctx:
        const = ctx.enter_context(tc.tile_pool(name="const", bufs=1))
        stage = ctx.enter_context(tc.tile_pool(name="stage", bufs=1))
        io = ctx.enter_context(tc.tile_pool(name="io", bufs=4))
        xsl = ctx.enter_context(tc.tile_pool(name="xsl", bufs=2))
        adjp = ctx.enter_context(tc.tile_pool(name="adjp", bufs=3))
        feat = ctx.enter_context(tc.tile_pool(name="feat", bufs=1))
        pbig = ctx.enter_context(tc.tile_pool(name="pbig", bufs=3, space="PSUM"))
        pout = ctx.enter_context(tc.tile_pool(name="pout", bufs=3, space="PSUM"))
        ptr = ctx.enter_context(tc.tile_pool(name="ptr", bufs=2, space="PSUM"))
        dram = ctx.enter_context(tc.tile_pool(name="dram", bufs=1, space="DRAM"))

        adjT = adjT_d[:]
        x_nm = x_nm_d[:]

        # ---------- degree -> dis on device (emitted first: feeds L1) -------
        def make_dis(name, dtensor, shape):
            # dis = sqrt(min(deg,1) * 1/max(deg,1)); all-DVE chain, one ACT sqrt
            dg = stage.tile(shape, f32, name=f"{name}_dg")
            nc.sync.dma_start(dg[:], dtensor[:])
            tmp = stage.tile(shape, f32, name=f"{name}_tmp")
            nc.vector.tensor_scalar_max(tmp[:], dg[:], 1.0)
            nc.vector.reciprocal(tmp[:], tmp[:])
            msk = stage.tile(shape, f32, name=f"{name}_msk")
            nc.vector.tensor_scalar_min(msk[:], dg[:], 1.0)
            nc.vector.tensor_tensor(tmp[:], tmp[:], msk[:], MUL)
            dis = const.tile(shape, f32, name=name)
            nc.scalar.activation(dis[:], tmp[:], AF.Sqrt)
            return dis

        dis_cols = make_dis("dis_cols", degc_d, [P, KT])  # dis over src (sigma order)
        dis_row = make_dis("dis_row", degr_d, [1, MLOC])  # dis over local dst

        # broadcast rows: ndis_bc[q, j] = -dis_row[j]; pdis_bc = +dis_row
        ones1f = const.tile([1, P], f32)
        nc.gpsimd.memset(ones1f[:], 1.0)
        ndis_row = const.tile([1, MLOC], f32)
        nc.vector.tensor_scalar_mul(ndis_row[:], dis_row[:], -1.0)
        ndis_bc = const.tile([P, MLOC], f32)
        pdis_bc = const.tile([P, MLOC], f16)
        for c0, cw in CH:
            pb = pout.tile([P, 512], f32, name="pb_bc", tag="po")
            nc.tensor.matmul(pb[:, :cw], ones1f[:], ndis_row[:, c0 : c0 + cw])
            nc.vector.tensor_copy(ndis_bc[:, c0 : c0 + cw], pb[:, :cw])
            nc.vector.tensor_scalar_mul(pdis_bc[:, c0 : c0 + cw], pb[:, :cw], -1.0)

        # ---------- x: scaled node-major pieces (slab DMA + DVE scale) ------
        xs1p = [feat.tile([P, GPP[p], D_IN], f16, name=f"xs1_{p}", tag=f"xy3_{p}") for p in range(3)]
        SLAB = 16
        for s0 in range(0, KT, SLAB):
            xslab = xsl.tile([P, SLAB, D_IN], f32, name="xslab")
            nc.sync.dma_start(xslab[:], x_nm[s0 : s0 + SLAB].rearrange("k q d -> q k d"))
            for k in range(SLAB):
                j = s0 + k
                p, r = _pos(j)
                nc.vector.tensor_scalar(
                    out=xs1p[p][:, r, :],
                    in0=xslab[:, k, :],
                    scalar1=dis_cols[:, j : j + 1],
                    scalar2=None,
                    op0=MUL,
                )

        # ---------- constants / weights ----------
        id16 = const.tile([P, P], f16)
        make_identity(nc, id16)
        id32 = const.tile([P, P], f32)
        make_identity(nc, id32)
        onesrow = const.tile([1, MLOC], f16)
        nc.gpsimd.memset(onesrow[:], 1.0)

        def load_cast(name, dtensor, shape):
            wf = stage.tile(shape, f32, name=f"{name}_f", tag="wst", bufs=2)
            nc.sync.dma_start(wf[:], dtensor[:])
            wh = const.tile(shape, f16, name=name)
            nc.vector.tensor_copy(wh[:], wf[:])
            return wh

        w01h = load_cast("w01h", w01_d, [P, D_HID])
        w11h = load_cast("w11h", w11_d, [P, D_HID])
        w02h = load_cast("w02h", w02_d[:].rearrange("b p w -> p b w"), [P, 2, D_HID])
        w12h = load_cast("w12h", w12_d[:].rearrange("b p w -> p b w"), [P, 2, D_HID])
        w03h = load_cast("w03h", w03_d[:].rearrange("b p w -> p b w"), [P, 2, D_OUT])
        w13h = load_cast("w13h", w13_d[:].rearrange("b p w -> p b w"), [P, 2, D_OUT])
        b1h = load_cast("b1h", b1_d, [1, D_HID])
        b2h = load_cast("b2h", b2_d, [1, D_HID])
        b3h = load_cast("b3h", b3_d, [1, D_OUT])

        xT16 = const.tile([P, MLOC], f16)
        xTf = stage.tile([P, MLOC], f32, name="xTf")
        nc.sync.dma_start(xTf[:], xT_d[:])
        nc.vector.tensor_copy(xT16[:], xTf[:])

        # ---------- chunk-outer adjacency matmul (slab-batched DMA) ----------
        KSLAB = 10  # source chunks per adjacency DMA (amortize ~1us issue cost)

        def big_chunk(ci, blocks):
            """Accumulate T^T[:, chunk ci] over all 80 source chunks.
            blocks: list of (feat_block_fn, ts_out_ap) pairs, one per 128-row
            feature block (d=256 runs both against one adjacency stream)."""
            c0, cw = CH[ci]
            accs = [pbig.tile([P, 512], f32, name=f"acc{b}", tag="acc") for b in range(len(blocks))]
            for s0 in range(0, KT, KSLAB):
                at = adjp.tile([P, KSLAB, 512], f8, name="at")
                nc.sync.dma_start(
                    at[:, :, :cw],
                    adjT[s0 : s0 + KSLAB, :, c0 : c0 + cw].rearrange("k q m -> q k m"),
                )
                for k in range(KSLAB):
                    j = s0 + k
                    for b, (fb, _) in enumerate(blocks):
                        nc.tensor.matmul(
                            accs[b][:, :cw], fb(j), at[:, k, :cw],
                            start=(j == 0), stop=(j == KT - 1),
                        )
            for b, (_, out_ap) in enumerate(blocks):
                nc.vector.tensor_tensor(out_ap, accs[b][:, :cw], ndis_bc[:, c0 : c0 + cw], MUL)

        # transpose feature-major blocks to node-major tiles + bounce + gather
        def emit_gather(ci, srcT, blocks, width, bounce, bounce_v, gathered, gout):
            c0, cw = CH[ci]
            ts = list(PIECE_TILES[ci])
            hst = io.tile([P, len(ts), width], f16, name="hst", tag="hst")
            for ti, t in enumerate(ts):
                for b in range(blocks):
                    ptt = ptr.tile([P, P], f16, name="ptt")
                    nc.tensor.transpose(ptt[:], srcT(b, t), id16[:])
                    nc.vector.tensor_copy(hst[:, ti, b * P : (b + 1) * P], ptt[:])
            nc.sync.dma_start(bounce_v[:, ts[0] : ts[0] + len(ts), :], hst[:])
            nc.gpsimd.collective_compute(
                "AllGather",
                mybir.AluOpType.bypass,
                replica_groups=[list(range(NCORES))],
                ins=[bounce[c0 : c0 + cw, :]],
                outs=[gathered[:]],
            )
            nc.sync.dma_start(gout[:], gathered[:].rearrange("(g q) d -> q g d", q=P))

        # ================= Layer 1 =================
        t1s = feat.tile([P, MLOC], f16)
        h1T = feat.tile([P, 2, MLOC], f16)
        h1sT = feat.tile([P, 2, MLOC], f16)
        h1s_bounce = dram.tile([MLOC, D_HID], f16, name="h1s_bounce")
        h1s_bounce_v = h1s_bounce[:].rearrange("(t q) d -> q t d", q=P)
        y2fp = [feat.tile([P, GPP[p], D_HID], f16, name=f"y2f_{p}") for p in range(3)]
        h1s_gath = [
            dram.tile([GPP[p] * P, D_HID], f16, name=f"h1s_gath{p}", addr_space="Shared")
            for p in range(3)
        ]

        for ci, (c0, cw) in enumerate(CH):
            cs = slice(c0, c0 + cw)
            big_chunk(
                ci,
                [(lambda j: xs1p[_pos(j)[0]][:, _pos(j)[1], :], t1s[:, cs])],
            )
            for bo in range(2):
                bs = slice(bo * P, (bo + 1) * P)
                po = pout.tile([P, 512], f32, name="po")
                nc.tensor.matmul(po[:, :cw], w01h[:, bs], xT16[:, cs], start=True, stop=False)
                nc.tensor.matmul(po[:, :cw], w11h[:, bs], t1s[:, cs], start=False, stop=False)
                nc.tensor.matmul(po[:, :cw], b1h[:, bs], onesrow[:, cs], start=False, stop=True)
                nc.vector.tensor_relu(h1T[:, bo, cs], po[:, :cw])
            for b in range(2):
                nc.vector.tensor_tensor(h1sT[:, b, cs], h1T[:, b, cs], pdis_bc[:, cs], MUL)
            emit_gather(
                ci,
                lambda b, t: h1sT[:, b, t * P : (t + 1) * P],
                2,
                D_HID,
                h1s_bounce,
                h1s_bounce_v,
                h1s_gath[ci],
                y2fp[ci][:],
            )

        # ================= Layer 2 =================
        t2s = feat.tile([P, 2, MLOC], f16)
        h2T = feat.tile([P, 2, MLOC], f16)
        h2sT = feat.tile([P, 2, MLOC], f16)
        y3T = feat.tile([P, MLOC], f16)
        y3_bounce = dram.tile([MLOC, D_OUT], f16, name="y3_bounce")
        y3_bounce_v = y3_bounce[:].rearrange("(t q) d -> q t d", q=P)
        y3fp = [feat.tile([P, GPP[p], D_OUT], f16, name=f"y3f_{p}", tag=f"xy3_{p}") for p in range(3)]
        y3_gath = [
            dram.tile([GPP[p] * P, D_OUT], f16, name=f"y3_gath{p}", addr_space="Shared")
            for p in range(3)
        ]

        for ci, (c0, cw) in enumerate(CH):
            cs = slice(c0, c0 + cw)
            big_chunk(
                ci,
                [
                    (lambda j, b=b: y2fp[_pos(j)[0]][:, _pos(j)[1], b * P : (b + 1) * P],
                     t2s[:, b, cs])
                    for b in range(2)
                ],
            )
            for bo in range(2):
                bs = slice(bo * P, (bo + 1) * P)
                po = pout.tile([P, 512], f32, name="po")
                nc.tensor.matmul(po[:, :cw], w02h[:, 0, bs], h1T[:, 0, cs], start=True, stop=False)
                nc.tensor.matmul(po[:, :cw], w02h[:, 1, bs], h1T[:, 1, cs], start=False, stop=False)
                nc.tensor.matmul(po[:, :cw], w12h[:, 0, bs], t2s[:, 0, cs], start=False, stop=False)
                nc.tensor.matmul(po[:, :cw], w12h[:, 1, bs], t2s[:, 1, cs], start=False, stop=False)
                nc.tensor.matmul(po[:, :cw], b2h[:, bs], onesrow[:, cs], start=False, stop=True)
                nc.vector.tensor_relu(h2T[:, bo, cs], po[:, :cw])
            for b in range(2):
                nc.vector.tensor_tensor(h2sT[:, b, cs], h2T[:, b, cs], pdis_bc[:, cs], MUL)
            # Y3 = (dis .* h2) @ W13, feature-major
            py = pout.tile([P, 512], f32, name="po")
            nc.tensor.matmul(py[:, :cw], w13h[:, 0, :], h2sT[:, 0, cs], start=True, stop=False)
            nc.tensor.matmul(py[:, :cw], w13h[:, 1, :], h2sT[:, 1, cs], start=False, stop=True)
            nc.vector.tensor_copy(y3T[:, cs], py[:, :cw])
            emit_gather(
                ci,
                lambda b, t: y3T[:, t * P : (t + 1) * P],
                1,
                D_OUT,
                y3_bounce,
                y3_bounce_v,
                y3_gath[ci],
                y3fp[ci][:],
            )

        # ================= Layer 3 =================
        t3s = feat.tile([P, MLOC], f16)
        outT = feat.tile([P, MLOC], f32)
        for ci, (c0, cw) in enumerate(CH):
            cs = slice(c0, c0 + cw)
            big_chunk(
                ci,
                [(lambda j: y3fp[_pos(j)[0]][:, _pos(j)[1], :], t3s[:, cs])],
            )
            po = pout.tile([P, 512], f32, name="po")
            nc.tensor.matmul(po[:, :cw], w03h[:, 0, :], h2T[:, 0, cs], start=True, stop=False)
            nc.tensor.matmul(po[:, :cw], w03h[:, 1, :], h2T[:, 1, cs], start=False, stop=False)
            nc.tensor.matmul(po[:, :cw], b3h[:], onesrow[:, cs], start=False, stop=False)
            # += T3s (identity-matmul add of the scaled Chebyshev term)
            nc.tensor.matmul(po[:, :cw], id16[:], t3s[:, cs], start=False, stop=True)
            nc.vector.tensor_copy(outT[:, cs], po[:, :cw])
            nc.sync.dma_start(out_d[:, cs], outT[:, cs])

    nc.compile()
    return nc


def _prep_inputs(x, edge_index, W01, W11, b1, W02, W12, b2, W03, W13, b3):
    f8 = ml_dtypes.float8_e4m3
    x = np.asarray(x, np.float32)
    ei = np.asarray(edge_index)
    src = ei[0].astype(np.int64)
    dst = ei[1].astype(np.int64)

    deg = np.bincount(src, minlength=NP).astype(np.float32)  # out-degree counts
    x_pad = np.zeros((NP, D_IN), np.float32)
    x_pad[:N] = x

    sig = np.asarray(SIGMA)
    x_nm = np.ascontiguousarray(x_pad.reshape(KT, P, D_IN)[sig])
    deg_cols = np.ascontiguousarray(deg.reshape(KT, P)[sig].T)

    common = {
        "x_nm": x_nm,
        "deg_cols": deg_cols,
        "w01": np.ascontiguousarray(np.asarray(W01, np.float32)),
        "w11": np.ascontiguousarray(np.asarray(W11, np.float32)),
        "w02": np.ascontiguousarray(np.asarray(W02, np.float32).reshape(2, P, D_HID)),
        "w12": np.ascontiguousarray(np.asarray(W12, np.float32).reshape(2, P, D_HID)),
        "w03": np.ascontiguousarray(np.asarray(W03, np.float32).reshape(2, P, D_OUT)),
        "w13": np.ascontiguousarray(np.asarray(W13, np.float32).reshape(2, P, D_OUT)),
        "b1r": np.asarray(b1, np.float32).reshape(1, D_HID),
        "b2r": np.asarray(b2, np.float32).reshape(1, D_HID),
        "b3r": np.asarray(b3, np.float32).reshape(1, D_OUT),
    }

    in_maps = []
    for c in range(NCORES):
        lo, hi = c * MLOC, (c + 1) * MLOC
        sel = (dst >= lo) & (dst < hi)
        idx = src[sel] * MLOC + (dst[sel] - lo)
        cnt = np.bincount(idx, minlength=NP * MLOC).astype(np.float32)
        adjT = np.ascontiguousarray(cnt.reshape(KT, P, MLOC)[sig]).astype(f8)
        m = dict(common)
        m["adjT"] = adjT
        m["xT_loc"] = np.ascontiguousarray(x_pad[lo:hi].T)
        m["deg_row"] = np.ascontiguousarray(deg[lo:hi].reshape(1, MLOC))
        in_maps.append(m)
    return in_maps


def kernel(x, edge_index, edge_type, W01, W11, b1, W02, W12, b2, W03, W13, b3):
    global LAST_RESULTS
    from concourse.bass_utils import run_bass_kernel_spmd

    if "nc" not in _CACHE:
        _CACHE["nc"] = _build_nc()
    nc = _CACHE["nc"]

    in_maps = _prep_inputs(x, edge_index, W01, W11, b1, W02, W12, b2, W03, W13, b3)
    res = run_bass_kernel_spmd(
        nc,
        in_maps,
        list(range(NCORES)),
        trace=bool(os.environ.get("BASS_TRACE")),
    )
    LAST_RESULTS = res
    shards = [res.results[c]["outT"].astype(np.float32).T for c in range(NCORES)]
    out = np.concatenate(shards, axis=0)[:N]
    return np.ascontiguousarray(out)


if __name__ == "__main__":
    _build_nc()
    print("build ok")


# revision 17
# speedup vs baseline: 1.8029x; 1.0666x over previous
"""ChebGCN (K=2, 3 layers) Trainium2 Bass kernel.

Strategy (1D graph/data parallel, dst-sharded):
  - Host: convert edge list -> dense adjacency COUNT strips AdjT[src, dst_local]
    per core (fp8 e4m3: small integer counts are exact), pad N 10000->10240,
    shard dst rows 1280/core. Pure format conversion; all FP math on device.
  - Device (SPMD on 8 cores):
      dis = sqrt(min(deg,1)/max(deg,1)) computed on device from integer counts.
      Per layer, the Chebyshev term  L_hat x = -D A D x  is a dense tensor-
      engine matmul  T^T = (dis .* X)^T @ AdjT  (fp16 x fp8), scaled by
      -dis_dst at PSUM evacuation. Dense W0/W1 matmuls run feature-major;
      layer outputs are PE-transposed to node-major only for the gather.
  - The adjacency matmul iterates dst-column chunks OUTER (512/512/256), so
    each chunk's result is ready early; its dense part + AllGather piece
    overlap the next chunk's accumulation. Source chunks are consumed in a
    host-side permutation (SIGMA) matching the gathered piece layout.
  - Layer 3 folds W13 before the gather (Y3 = (dis.*h2) @ W13), halving the
    final dense-adjacency matmul width.

kernel(**inputs) takes the FULL unsharded inputs and returns the FULL output.
"""

import os
import sys

sys.path.insert(0, "/opt/trn_rl_repo")

import numpy as np
import ml_dtypes

N = 10000
NP = 10240           # padded node count
NCORES = 8
MLOC = NP // NCORES  # 1280 dst rows per core
P = 128
KT = NP // P         # 80 source chunks of 128
TPC = MLOC // P      # 10 dst tiles per core
D_IN, D_HID, D_OUT = 128, 256, 128
# dst column chunks == gather pieces (psum-bank sized)
CH = [(0, 512), (512, 512), (1024, 256)]
PIECE_TILES = [range(0, 4), range(4, 8), range(8, 10)]
GPP = [32, 32, 16]        # global chunks per piece (8 cores x tiles)
PSTART = [0, 32, 64]      # first sigma position of each piece

# sigma position -> global chunk: piece-major, then (core, tile)
SIGMA = [c * TPC + t for ts in PIECE_TILES for c in range(NCORES) for t in ts]


def _pos(j):
    """sigma position -> (piece index, slot within piece)"""
    for pi in range(3):
        if j < PSTART[pi] + GPP[pi]:
            return pi, j - PSTART[pi]
    raise ValueError(j)


_CACHE = {}
LAST_RESULTS = None  # BassKernelResults of the most recent run (for profiling)


def _build_nc():
    from contextlib import ExitStack

    import concourse.bass as bass
    import concourse.tile as tile
    from concourse import bacc, mybir
    from concourse.masks import make_identity

    f32 = mybir.dt.float32
    f16 = mybir.dt.float16
    f8 = mybir.dt.float8e4
    AF = mybir.ActivationFunctionType
    MUL = mybir.AluOpType.mult

    nc = bacc.Bacc(trn_type="TRN2", num_devices=NCORES)

    adjT_d = nc.dram_tensor("adjT", [KT, P, MLOC], f8, kind="ExternalInput")
    x_nm_d = nc.dram_tensor("x_nm", [KT, P, D_IN], f32, kind="ExternalInput")
    xT_d = nc.dram_tensor("xT_loc", [P, MLOC], f32, kind="ExternalInput")
    degc_d = nc.dram_tensor("deg_cols", [P, KT], f32, kind="ExternalInput")
    degr_d = nc.dram_tensor("deg_row", [1, MLOC], f32, kind="ExternalInput")
    w01_d = nc.dram_tensor("w01", [P, D_HID], f32, kind="ExternalInput")
    w11_d = nc.dram_tensor("w11", [P, D_HID], f32, kind="ExternalInput")
    w02_d = nc.dram_tensor("w02", [2, P, D_HID], f32, kind="ExternalInput")
    w12_d = nc.dram_tensor("w12", [2, P, D_HID], f32, kind="ExternalInput")
    w03_d = nc.dram_tensor("w03", [2, P, D_OUT], f32, kind="ExternalInput")
    w13_d = nc.dram_tensor("w13", [2, P, D_OUT], f32, kind="ExternalInput")
    b1_d = nc.dram_tensor("b1r", [1, D_HID], f32, kind="ExternalInput")
    b2_d = nc.dram_tensor("b2r", [1, D_HID], f32, kind="ExternalInput")
    b3_d = nc.dram_tensor("b3r", [1, D_OUT], f32, kind="ExternalInput")
    out_d = nc.dram_tensor("outT", [P, MLOC], f32, kind="ExternalOutput")

    with tile.TileContext(nc) as tc, ExitStack() as ctx:
        const = ctx.enter_context(tc.tile_pool(name="const", bufs=1))
        stage = ctx.enter_context(tc.tile_pool(name="stage", bufs=1))
        io = ctx.enter_context(tc.tile_pool(name="io", bufs=4))
        xsl = ctx.enter_context(tc.tile_pool(name="xsl", bufs=2))
        adjp = ctx.enter_context(tc.tile_pool(name="adjp", bufs=3))
        feat = ctx.enter_context(tc.tile_pool(name="feat", bufs=1))
        pbig = ctx.enter_context(tc.tile_pool(name="pbig", bufs=3, space="PSUM"))
        pout = ctx.enter_context(tc.tile_pool(name="pout", bufs=3, space="PSUM"))
        ptr = ctx.enter_context(tc.tile_pool(name="ptr", bufs=2, space="PSUM"))
        dram = ctx.enter_context(tc.tile_pool(name="dram", bufs=1, space="DRAM"))

        adjT = adjT_d[:]
        x_nm = x_nm_d[:]

        # ---------- degree -> dis on device (emitted first: feeds L1) -------
        def make_dis(name, dtensor, shape):
            # dis = sqrt(min(deg,1) * 1/max(deg,1)); all-DVE chain, one ACT sqrt
            dg = stage.tile(shape, f32, name=f"{name}_dg")
            nc.sync.dma_start(dg[:], dtensor[:])
            tmp = stage.tile(shape, f32, name=f"{name}_tmp")
            nc.vector.tensor_scalar_max(tmp[:], dg[:], 1.0)
            nc.vector.reciprocal(tmp[:], tmp[:])
            msk = stage.tile(shape, f32, name=f"{name}_msk")
            nc.vector.tensor_scalar_min(msk[:], dg[:], 1.0)
            nc.vector.tensor_tensor(tmp[:], tmp[:], msk[:], MUL)
            dis = const.tile(shape, f32, name=name)
            nc.scalar.activation(dis[:], tmp[:], AF.Sqrt)
            return dis

        dis_cols = make_dis("dis_cols", degc_d, [P, KT])  # dis over src (sigma order)
        dis_row = make_dis("dis_row", degr_d, [1, MLOC])  # dis over local dst

        # broadcast rows: ndis_bc[q, j] = -dis_row[j]; pdis_bc = +dis_row
        ones1f = const.tile([1, P], f32)
        nc.gpsimd.memset(ones1f[:], 1.0)
        ndis_row = const.tile([1, MLOC], f32)
        nc.vector.tensor_scalar_mul(ndis_row[:], dis_row[:], -1.0)
        ndis_bc = const.tile([P, MLOC], f32)
        pdis_bc = const.tile([P, MLOC], f16)
        for c0, cw in CH:
            pb = pout.tile([P, 512], f32, name="pb_bc", tag="po")
            nc.tensor.matmul(pb[:, :cw], ones1f[:], ndis_row[:, c0 : c0 + cw])
            nc.vector.tensor_copy(ndis_bc[:, c0 : c0 + cw], pb[:, :cw])
            nc.vector.tensor_scalar_mul(pdis_bc[:, c0 : c0 + cw], pb[:, :cw], -1.0)

        # ---------- x: scaled node-major pieces (slab DMA + DVE scale) ------
        xs1p = [feat.tile([P, GPP[p], D_IN], f16, name=f"xs1_{p}", tag=f"xy3_{p}") for p in range(3)]
        SLAB = 16
        for s0 in range(0, KT, SLAB):
            xslab = xsl.tile([P, SLAB, D_IN], f32, name="xslab")
            nc.sync.dma_start(xslab[:], x_nm[s0 : s0 + SLAB].rearrange("k q d -> q k d"))
            for k in range(SLAB):
                j = s0 + k
                p, r = _pos(j)
                nc.vector.tensor_scalar(
                    out=xs1p[p][:, r, :],
                    in0=xslab[:, k, :],
                    scalar1=dis_cols[:, j : j + 1],
                    scalar2=None,
                    op0=MUL,
                )

        # ---------- constants / weights ----------
        id16 = const.tile([P, P], f16)
        make_identity(nc, id16)
        id32 = const.tile([P, P], f32)
        make_identity(nc, id32)
        onesrow = const.tile([1, MLOC], f16)
        nc.gpsimd.memset(onesrow[:], 1.0)

        def load_cast(name, dtensor, shape):
            wf = stage.tile(shape, f32, name=f"{name}_f", tag="wst", bufs=2)
            nc.scalar.dma_start(wf[:], dtensor[:])
            wh = const.tile(shape, f16, name=name)
            nc.vector.tensor_copy(wh[:], wf[:])
            return wh

        w01h = load_cast("w01h", w01_d, [P, D_HID])
        w11h = load_cast("w11h", w11_d, [P, D_HID])
        w02h = load_cast("w02h", w02_d[:].rearrange("b p w -> p b w"), [P, 2, D_HID])
        w12h = load_cast("w12h", w12_d[:].rearrange("b p w -> p b w"), [P, 2, D_HID])
        w03h = load_cast("w03h", w03_d[:].rearrange("b p w -> p b w"), [P, 2, D_OUT])
        w13h = load_cast("w13h", w13_d[:].rearrange("b p w -> p b w"), [P, 2, D_OUT])
        b1h = load_cast("b1h", b1_d, [1, D_HID])
        b2h = load_cast("b2h", b2_d, [1, D_HID])
        b3h = load_cast("b3h", b3_d, [1, D_OUT])

        xT16 = const.tile([P, MLOC], f16)
        xTf = stage.tile([P, MLOC], f32, name="xTf")
        nc.scalar.dma_start(xTf[:], xT_d[:])
        nc.vector.tensor_copy(xT16[:], xTf[:])

        # ---------- chunk-outer adjacency matmul (slab-batched DMA) ----------
        KSLAB = 10  # source chunks per adjacency DMA (amortize ~1us issue cost)

        def big_chunk(ci, blocks):
            """Accumulate T^T[:, chunk ci] over all 80 source chunks.
            blocks: list of (feat_block_fn, ts_out_ap) pairs, one per 128-row
            feature block (d=256 runs both against one adjacency stream)."""
            c0, cw = CH[ci]
            accs = [pbig.tile([P, 512], f32, name=f"acc{b}", tag="acc") for b in range(len(blocks))]
            for s0 in range(0, KT, KSLAB):
                at = adjp.tile([P, KSLAB, 512], f8, name="at")
                nc.sync.dma_start(
                    at[:, :, :cw],
                    adjT[s0 : s0 + KSLAB, :, c0 : c0 + cw].rearrange("k q m -> q k m"),
                )
                for k in range(KSLAB):
                    j = s0 + k
                    for b, (fb, _) in enumerate(blocks):
                        nc.tensor.matmul(
                            accs[b][:, :cw], fb(j), at[:, k, :cw],
                            start=(j == 0), stop=(j == KT - 1),
                        )
            for b, (_, out_ap) in enumerate(blocks):
                nc.vector.tensor_tensor(out_ap, accs[b][:, :cw], ndis_bc[:, c0 : c0 + cw], MUL)

        # transpose feature-major blocks to node-major tiles + bounce + gather
        def emit_gather(ci, srcT, blocks, width, bounce, bounce_v, gathered, gout):
            c0, cw = CH[ci]
            ts = list(PIECE_TILES[ci])
            hst = io.tile([P, len(ts), width], f16, name="hst", tag="hst")
            for ti, t in enumerate(ts):
                for b in range(blocks):
                    ptt = ptr.tile([P, P], f16, name="ptt")
                    nc.tensor.transpose(ptt[:], srcT(b, t), id16[:])
                    nc.vector.tensor_copy(hst[:, ti, b * P : (b + 1) * P], ptt[:])
            nc.scalar.dma_start(bounce_v[:, ts[0] : ts[0] + len(ts), :], hst[:])
            nc.gpsimd.collective_compute(
                "AllGather",
                mybir.AluOpType.bypass,
                replica_groups=[list(range(NCORES))],
                ins=[bounce[c0 : c0 + cw, :]],
                outs=[gathered[:]],
            )
            nc.gpsimd.dma_start(gout[:], gathered[:].rearrange("(g q) d -> q g d", q=P))

        # ================= Layer 1 =================
        t1s = feat.tile([P, MLOC], f16)
        h1T = feat.tile([P, 2, MLOC], f16)
        h1sT = feat.tile([P, 2, MLOC], f16)
        h1s_bounce = dram.tile([MLOC, D_HID], f16, name="h1s_bounce")
        h1s_bounce_v = h1s_bounce[:].rearrange("(t q) d -> q t d", q=P)
        y2fp = [feat.tile([P, GPP[p], D_HID], f16, name=f"y2f_{p}") for p in range(3)]
        h1s_gath = [
            dram.tile([GPP[p] * P, D_HID], f16, name=f"h1s_gath{p}", addr_space="Shared")
            for p in range(3)
        ]

        for ci, (c0, cw) in enumerate(CH):
            cs = slice(c0, c0 + cw)
            big_chunk(
                ci,
                [(lambda j: xs1p[_pos(j)[0]][:, _pos(j)[1], :], t1s[:, cs])],
            )
            for bo in range(2):
                bs = slice(bo * P, (bo + 1) * P)
                po = pout.tile([P, 512], f32, name="po")
                nc.tensor.matmul(po[:, :cw], w01h[:, bs], xT16[:, cs], start=True, stop=False)
                nc.tensor.matmul(po[:, :cw], w11h[:, bs], t1s[:, cs], start=False, stop=False)
                nc.tensor.matmul(po[:, :cw], b1h[:, bs], onesrow[:, cs], start=False, stop=True)
                nc.vector.tensor_relu(h1T[:, bo, cs], po[:, :cw])
            for b in range(2):
                nc.vector.tensor_tensor(h1sT[:, b, cs], h1T[:, b, cs], pdis_bc[:, cs], MUL)
            emit_gather(
                ci,
                lambda b, t: h1sT[:, b, t * P : (t + 1) * P],
                2,
                D_HID,
                h1s_bounce,
                h1s_bounce_v,
                h1s_gath[ci],
                y2fp[ci][:],
            )

        # ================= Layer 2 =================
        t2s = feat.tile([P, 2, MLOC], f16)
        h2T = feat.tile([P, 2, MLOC], f16)
        h2sT = feat.tile([P, 2, MLOC], f16)
        y3T = feat.tile([P, MLOC], f16)
        y3_bounce = dram.tile([MLOC, D_OUT], f16, name="y3_bounce")
        y3_bounce_v = y3_bounce[:].rearrange("(t q) d -> q t d", q=P)
        y3fp = [feat.tile([P, GPP[p], D_OUT], f16, name=f"y3f_{p}", tag=f"xy3_{p}") for p in range(3)]
        y3_gath = [
            dram.tile([GPP[p] * P, D_OUT], f16, name=f"y3_gath{p}", addr_space="Shared")
            for p in range(3)
        ]

        for ci, (c0, cw) in enumerate(CH):
            cs = slice(c0, c0 + cw)
            big_chunk(
                ci,
                [
                    (lambda j, b=b: y2fp[_pos(j)[0]][:, _pos(j)[1], b * P : (b + 1) * P],
                     t2s[:, b, cs])
                    for b in range(2)
                ],
            )
            for bo in range(2):
                bs = slice(bo * P, (bo + 1) * P)
                po = pout.tile([P, 512], f32, name="po")
                nc.tensor.matmul(po[:, :cw], w02h[:, 0, bs], h1T[:, 0, cs], start=True, stop=False)
                nc.tensor.matmul(po[:, :cw], w02h[:, 1, bs], h1T[:, 1, cs], start=False, stop=False)
                nc.tensor.matmul(po[:, :cw], w12h[:, 0, bs], t2s[:, 0, cs], start=False, stop=False)
                nc.tensor.matmul(po[:, :cw], w12h[:, 1, bs], t2s[:, 1, cs], start=False, stop=False)
                nc.tensor.matmul(po[:, :cw], b2h[:, bs], onesrow[:, cs], start=False, stop=True)
                nc.vector.tensor_relu(h2T[:, bo, cs], po[:, :cw])
            for b in range(2):
                nc.vector.tensor_tensor(h2sT[:, b, cs], h2T[:, b, cs], pdis_bc[:, cs], MUL)
            # Y3 = (dis .* h2) @ W13, feature-major
            py = pout.tile([P, 512], f32, name="po")
            nc.tensor.matmul(py[:, :cw], w13h[:, 0, :], h2sT[:, 0, cs], start=True, stop=False)
            nc.tensor.matmul(py[:, :cw], w13h[:, 1, :], h2sT[:, 1, cs], start=False, stop=True)
            nc.vector.tensor_copy(y3T[:, cs], py[:, :cw])
            emit_gather(
                ci,
                lambda b, t: y3T[:, t * P : (t + 1) * P],
                1,
                D_OUT,
                y3_bounce,
                y3_bounce_v,
                y3_gath[ci],
                y3fp[ci][:],
            )

        # ================= Layer 3 =================
        t3s = feat.tile([P, MLOC], f16)
        outT = feat.tile([P, MLOC], f32)
        for ci, (c0, cw) in enumerate(CH):
            cs = slice(c0, c0 + cw)
            big_chunk(
                ci,
                [(lambda j: y3fp[_pos(j)[0]][:, _pos(j)[1], :], t3s[:, cs])],
            )
            po = pout.tile([P, 512], f32, name="po")
            nc.tensor.matmul(po[:, :cw], w03h[:, 0, :], h2T[:, 0, cs], start=True, stop=False)
            nc.tensor.matmul(po[:, :cw], w03h[:, 1, :], h2T[:, 1, cs], start=False, stop=False)
            nc.tensor.matmul(po[:, :cw], b3h[:], onesrow[:, cs], start=False, stop=False)
            # += T3s (identity-matmul add of the scaled Chebyshev term)
            nc.tensor.matmul(po[:, :cw], id16[:], t3s[:, cs], start=False, stop=True)
            nc.vector.tensor_copy(outT[:, cs], po[:, :cw])
            nc.scalar.dma_start(out_d[:, cs], outT[:, cs])

    nc.compile()
    return nc


def _prep_inputs(x, edge_index, W01, W11, b1, W02, W12, b2, W03, W13, b3):
    f8 = ml_dtypes.float8_e4m3
    x = np.asarray(x, np.float32)
    ei = np.asarray(edge_index)
    src = ei[0].astype(np.int64)
    dst = ei[1].astype(np.int64)

    deg = np.bincount(src, minlength=NP).astype(np.float32)  # out-degree counts
    x_pad = np.zeros((NP, D_IN), np.float32)
    x_pad[:N] = x

    sig = np.asarray(SIGMA)
    x_nm = np.ascontiguousarray(x_pad.reshape(KT, P, D_IN)[sig])
    deg_cols = np.ascontiguousarray(deg.reshape(KT, P)[sig].T)

    common = {
        "x_nm": x_nm,
        "deg_cols": deg_cols,
        "w01": np.ascontiguousarray(np.asarray(W01, np.float32)),
        "w11": np.ascontiguousarray(np.asarray(W11, np.float32)),
        "w02": np.ascontiguousarray(np.asarray(W02, np.float32).reshape(2, P, D_HID)),
        "w12": np.ascontiguousarray(np.asarray(W12, np.float32).reshape(2, P, D_HID)),
        "w03": np.ascontiguousarray(np.asarray(W03, np.float32).reshape(2, P, D_OUT)),
        "w13": np.ascontiguousarray(np.asarray(W13, np.float32).reshape(2, P, D_OUT)),
        "b1r": np.asarray(b1, np.float32).reshape(1, D_HID),
        "b2r": np.asarray(b2, np.float32).reshape(1, D_HID),
        "b3r": np.asarray(b3, np.float32).reshape(1, D_OUT),
    }

    in_maps = []
    for c in range(NCORES):
        lo, hi = c * MLOC, (c + 1) * MLOC
        sel = (dst >= lo) & (dst < hi)
        idx = src[sel] * MLOC + (dst[sel] - lo)
        cnt = np.bincount(idx, minlength=NP * MLOC).astype(np.float32)
        adjT = np.ascontiguousarray(cnt.reshape(KT, P, MLOC)[sig]).astype(f8)
        m = dict(common)
        m["adjT"] = adjT
        m["xT_loc"] = np.ascontiguousarray(x_pad[lo:hi].T)
        m["deg_row"] = np.ascontiguousarray(deg[lo:hi].reshape(1, MLOC))
        in_maps.append(m)
    return in_maps


def kernel(x, edge_index, edge_type, W01, W11, b1, W02, W12, b2, W03, W13, b3):
    global LAST_RESULTS
    from concourse.bass_utils import run_bass_kernel_spmd

    if "nc" not in _CACHE:
        _CACHE["nc"] = _build_nc()
    nc = _CACHE["nc"]

    in_maps = _prep_inputs(x, edge_index, W01, W11, b1, W02, W12, b2, W03, W13, b3)
    res = run_bass_kernel_spmd(
        nc,
        in_maps,
        list(range(NCORES)),
        trace=bool(os.environ.get("BASS_TRACE")),
    )
    LAST_RESULTS = res
    shards = [res.results[c]["outT"].astype(np.float32).T for c in range(NCORES)]
    out = np.concatenate(shards, axis=0)[:N]
    return np.ascontiguousarray(out)


if __name__ == "__main__":
    _build_nc()
    print("build ok")


# revision 20
# speedup vs baseline: 1.8870x; 1.0466x over previous
"""ChebGCN (K=2, 3 layers) Trainium2 Bass kernel.

Strategy (1D graph/data parallel, dst-sharded):
  - Host: convert edge list -> dense adjacency COUNT strips AdjT[src, dst_local]
    per core (fp8 e4m3: small integer counts are exact), pad N 10000->10240,
    shard dst rows 1280/core. Pure format conversion; all FP math on device.
  - Device (SPMD on 8 cores):
      dis = sqrt(min(deg,1)/max(deg,1)) computed on device from integer counts.
      Per layer, the Chebyshev term  L_hat x = -D A D x  is a dense tensor-
      engine matmul  T^T = (dis .* X)^T @ AdjT  (fp16 x fp8), scaled by
      -dis_dst at PSUM evacuation. Dense W0/W1 matmuls run feature-major;
      layer outputs are PE-transposed to node-major only for the gather.
  - The adjacency matmul iterates dst-column chunks OUTER (512/512/256), so
    each chunk's result is ready early; its dense part + AllGather piece
    overlap the next chunk's accumulation. Source chunks are consumed in a
    host-side permutation (SIGMA) matching the gathered piece layout.
  - Layer 3 folds W13 before the gather (Y3 = (dis.*h2) @ W13), halving the
    final dense-adjacency matmul width.

kernel(**inputs) takes the FULL unsharded inputs and returns the FULL output.
"""

import os
import sys

sys.path.insert(0, "/opt/trn_rl_repo")

import numpy as np
import ml_dtypes

N = 10000
NP = 10240           # padded node count
NCORES = 8
MLOC = NP // NCORES  # 1280 dst rows per core
P = 128
KT = NP // P         # 80 source chunks of 128
TPC = MLOC // P      # 10 dst tiles per core
D_IN, D_HID, D_OUT = 128, 256, 128
# dst column chunks == gather pieces (psum-bank sized)
CH = [(0, 512), (512, 512), (1024, 256)]
PIECE_TILES = [range(0, 4), range(4, 8), range(8, 10)]
GPP = [32, 32, 16]        # global chunks per piece (8 cores x tiles)
PSTART = [0, 32, 64]      # first sigma position of each piece

# sigma position -> global chunk: piece-major, then (core, tile)
SIGMA = [c * TPC + t for ts in PIECE_TILES for c in range(NCORES) for t in ts]


def _pos(j):
    """sigma position -> (piece index, slot within piece)"""
    for pi in range(3):
        if j < PSTART[pi] + GPP[pi]:
            return pi, j - PSTART[pi]
    raise ValueError(j)


_CACHE = {}
LAST_RESULTS = None  # BassKernelResults of the most recent run (for profiling)


def _enable_ldw_opt():
    """compile_bir_kernel hardcodes --enable-ldw-opt=false; turn it on for this
    kernel's compile (allows LDWEIGHTS fast-load/background-buffer codegen)."""
    from concourse import bass_utils as bu

    if getattr(bu, "_ldw_opt_patched", False):
        return
    orig = bu.run_command

    def run_command_ldw(cmd, **kw):
        if isinstance(cmd, list):
            cmd = ["--enable-ldw-opt=true" if c == "--enable-ldw-opt=false" else c for c in cmd]
        return orig(cmd, **kw)

    bu.run_command = run_command_ldw
    bu._ldw_opt_patched = True


def _build_nc():
    from contextlib import ExitStack

    import concourse.bass as bass
    import concourse.tile as tile
    from concourse import bacc, mybir
    from concourse.masks import make_identity

    f32 = mybir.dt.float32
    f16 = mybir.dt.float16
    f8 = mybir.dt.float8e4
    AF = mybir.ActivationFunctionType
    MUL = mybir.AluOpType.mult

    nc = bacc.Bacc(trn_type="TRN2", num_devices=NCORES)

    adjT_d = nc.dram_tensor("adjT", [KT, P, MLOC], f8, kind="ExternalInput")
    x_nm_d = nc.dram_tensor("x_nm", [KT, P, D_IN], f16, kind="ExternalInput")
    xT_d = nc.dram_tensor("xT_loc", [P, MLOC], f32, kind="ExternalInput")
    degc_d = nc.dram_tensor("deg_cols", [P, KT], f32, kind="ExternalInput")
    degr_d = nc.dram_tensor("deg_row", [1, MLOC], f32, kind="ExternalInput")
    w01_d = nc.dram_tensor("w01", [P, D_HID], f32, kind="ExternalInput")
    w11_d = nc.dram_tensor("w11", [P, D_HID], f32, kind="ExternalInput")
    w02_d = nc.dram_tensor("w02", [2, P, D_HID], f32, kind="ExternalInput")
    w12_d = nc.dram_tensor("w12", [2, P, D_HID], f32, kind="ExternalInput")
    w03_d = nc.dram_tensor("w03", [2, P, D_OUT], f32, kind="ExternalInput")
    w13_d = nc.dram_tensor("w13", [2, P, D_OUT], f32, kind="ExternalInput")
    b1_d = nc.dram_tensor("b1r", [1, D_HID], f32, kind="ExternalInput")
    b2_d = nc.dram_tensor("b2r", [1, D_HID], f32, kind="ExternalInput")
    b3_d = nc.dram_tensor("b3r", [1, D_OUT], f32, kind="ExternalInput")
    out_d = nc.dram_tensor("outT", [P, MLOC], f32, kind="ExternalOutput")

    with tile.TileContext(nc) as tc, ExitStack() as ctx:
        const = ctx.enter_context(tc.tile_pool(name="const", bufs=1))
        stage = ctx.enter_context(tc.tile_pool(name="stage", bufs=1))
        io = ctx.enter_context(tc.tile_pool(name="io", bufs=4))
        xsl = ctx.enter_context(tc.tile_pool(name="xsl", bufs=2))
        adjp = ctx.enter_context(tc.tile_pool(name="adjp", bufs=3))
        feat = ctx.enter_context(tc.tile_pool(name="feat", bufs=1))
        pbig = ctx.enter_context(tc.tile_pool(name="pbig", bufs=3, space="PSUM"))
        pout = ctx.enter_context(tc.tile_pool(name="pout", bufs=3, space="PSUM"))
        ptr = ctx.enter_context(tc.tile_pool(name="ptr", bufs=2, space="PSUM"))
        dram = ctx.enter_context(tc.tile_pool(name="dram", bufs=1, space="DRAM"))

        adjT = adjT_d[:]
        x_nm = x_nm_d[:]

        # ---------- degree -> dis on device (emitted first: feeds L1) -------
        def make_dis(name, dtensor, shape):
            # dis = sqrt(min(deg,1) * 1/max(deg,1)); all-DVE chain, one ACT sqrt
            dg = stage.tile(shape, f32, name=f"{name}_dg")
            nc.sync.dma_start(dg[:], dtensor[:])
            tmp = stage.tile(shape, f32, name=f"{name}_tmp")
            nc.vector.tensor_scalar_max(tmp[:], dg[:], 1.0)
            nc.vector.reciprocal(tmp[:], tmp[:])
            msk = stage.tile(shape, f32, name=f"{name}_msk")
            nc.vector.tensor_scalar_min(msk[:], dg[:], 1.0)
            nc.vector.tensor_tensor(tmp[:], tmp[:], msk[:], MUL)
            dis = const.tile(shape, f32, name=name)
            nc.scalar.activation(dis[:], tmp[:], AF.Sqrt)
            return dis

        dis_cols = make_dis("dis_cols", degc_d, [P, KT])  # dis over src (sigma order)
        dis_row = make_dis("dis_row", degr_d, [1, MLOC])  # dis over local dst

        # tiny warmup AllGather: absorbs the CC stream start cost during L1
        cc_warm_in = dram.tile([8, 4], f32, name="cc_warm_in")
        cc_warm_out = dram.tile([64, 4], f32, name="cc_warm_out", addr_space="Shared")
        nc.sync.dma_start(cc_warm_in[:], degr_d[:, 0:32].rearrange("o (a b) -> (o a) b", b=4))
        nc.gpsimd.collective_compute(
            "AllGather",
            mybir.AluOpType.bypass,
            replica_groups=[list(range(NCORES))],
            ins=[cc_warm_in[:]],
            outs=[cc_warm_out[:]],
        )

        # broadcast rows: ndis_bc[q, j] = -dis_row[j]; pdis_bc = +dis_row
        ones1f = const.tile([1, P], f32)
        nc.gpsimd.memset(ones1f[:], 1.0)
        ndis_row = const.tile([1, MLOC], f32)
        nc.vector.tensor_scalar_mul(ndis_row[:], dis_row[:], -1.0)
        ndis_bc = const.tile([P, MLOC], f32)
        pdis_bc = const.tile([P, MLOC], f16)
        for c0, cw in CH:
            pb = pout.tile([P, 512], f32, name="pb_bc", tag="po")
            nc.tensor.matmul(pb[:, :cw], ones1f[:], ndis_row[:, c0 : c0 + cw])
            nc.vector.tensor_copy(ndis_bc[:, c0 : c0 + cw], pb[:, :cw])
            nc.vector.tensor_scalar_mul(pdis_bc[:, c0 : c0 + cw], pb[:, :cw], -1.0)

        # ---------- x: scaled node-major pieces (slab DMA + DVE scale) ------
        xs1p = [feat.tile([P, GPP[p], D_IN], f16, name=f"xs1_{p}", tag=f"xy3_{p}") for p in range(3)]
        SLAB = 16
        for s0 in range(0, KT, SLAB):
            xslab = xsl.tile([P, SLAB, D_IN], f16, name="xslab")
            nc.sync.dma_start(xslab[:], x_nm[s0 : s0 + SLAB].rearrange("k q d -> q k d"))
            for k in range(SLAB):
                j = s0 + k
                p, r = _pos(j)
                nc.vector.tensor_scalar(
                    out=xs1p[p][:, r, :],
                    in0=xslab[:, k, :],
                    scalar1=dis_cols[:, j : j + 1],
                    scalar2=None,
                    op0=MUL,
                )

        # ---------- constants / weights ----------
        id16 = const.tile([P, P], f16)
        make_identity(nc, id16)
        id32 = const.tile([P, P], f32)
        make_identity(nc, id32)
        onesrow = const.tile([1, MLOC], f16)
        nc.gpsimd.memset(onesrow[:], 1.0)

        def load_cast(name, dtensor, shape):
            wf = stage.tile(shape, f32, name=f"{name}_f", tag="wst", bufs=2)
            nc.scalar.dma_start(wf[:], dtensor[:])
            wh = const.tile(shape, f16, name=name)
            nc.vector.tensor_copy(wh[:], wf[:])
            return wh

        w01h = load_cast("w01h", w01_d, [P, D_HID])
        w11h = load_cast("w11h", w11_d, [P, D_HID])
        w02h = load_cast("w02h", w02_d[:].rearrange("b p w -> p b w"), [P, 2, D_HID])
        w12h = load_cast("w12h", w12_d[:].rearrange("b p w -> p b w"), [P, 2, D_HID])
        w03h = load_cast("w03h", w03_d[:].rearrange("b p w -> p b w"), [P, 2, D_OUT])
        w13h = load_cast("w13h", w13_d[:].rearrange("b p w -> p b w"), [P, 2, D_OUT])
        b1h = load_cast("b1h", b1_d, [1, D_HID])
        b2h = load_cast("b2h", b2_d, [1, D_HID])
        b3h = load_cast("b3h", b3_d, [1, D_OUT])

        xT16 = const.tile([P, MLOC], f16)
        xTf = stage.tile([P, MLOC], f32, name="xTf")
        nc.scalar.dma_start(xTf[:], xT_d[:])
        nc.vector.tensor_copy(xT16[:], xTf[:])

        # ---------- chunk-outer adjacency matmul (slab-batched DMA) ----------
        KSLAB = 10  # source chunks per adjacency DMA (amortize ~1us issue cost)

        def big_chunk(ci, blocks):
            """Accumulate T^T[:, chunk ci] over all 80 source chunks.
            blocks: list of (feat_block_fn, ts_out_ap) pairs, one per 128-row
            feature block (d=256 runs both against one adjacency stream)."""
            c0, cw = CH[ci]
            accs = [pbig.tile([P, 512], f32, name=f"acc{b}", tag="acc") for b in range(len(blocks))]
            for s0 in range(0, KT, KSLAB):
                at = adjp.tile([P, KSLAB, 512], f8, name="at")
                nc.sync.dma_start(
                    at[:, :, :cw],
                    adjT[s0 : s0 + KSLAB, :, c0 : c0 + cw].rearrange("k q m -> q k m"),
                )
                for k in range(KSLAB):
                    j = s0 + k
                    for b, (fb, _) in enumerate(blocks):
                        nc.tensor.matmul(
                            accs[b][:, :cw], fb(j), at[:, k, :cw],
                            start=(j == 0), stop=(j == KT - 1),
                        )
            for b, (_, out_ap) in enumerate(blocks):
                nc.vector.tensor_tensor(out_ap, accs[b][:, :cw], ndis_bc[:, c0 : c0 + cw], MUL)

        # transpose feature-major blocks to node-major tiles + bounce + gather
        def emit_gather(ci, srcT, blocks, width, bounce, bounce_v, gathered, gout):
            c0, cw = CH[ci]
            ts = list(PIECE_TILES[ci])
            hst = io.tile([P, len(ts), width], f16, name="hst", tag="hst")
            for ti, t in enumerate(ts):
                for b in range(blocks):
                    ptt = ptr.tile([P, P], f16, name="ptt")
                    nc.tensor.transpose(ptt[:], srcT(b, t), id16[:])
                    nc.vector.tensor_copy(hst[:, ti, b * P : (b + 1) * P], ptt[:])
            nc.scalar.dma_start(bounce_v[:, ts[0] : ts[0] + len(ts), :], hst[:])
            nc.gpsimd.collective_compute(
                "AllGather",
                mybir.AluOpType.bypass,
                replica_groups=[list(range(NCORES))],
                ins=[bounce[c0 : c0 + cw, :]],
                outs=[gathered[:]],
            )
            nc.gpsimd.dma_start(gout[:], gathered[:].rearrange("(g q) d -> q g d", q=P))

        # ================= Layer 1 =================
        t1s = feat.tile([P, MLOC], f16)
        h1T = feat.tile([P, 2, MLOC], f16)
        h1sT = feat.tile([P, 2, MLOC], f16)
        h1s_bounce = dram.tile([MLOC, D_HID], f16, name="h1s_bounce")
        h1s_bounce_v = h1s_bounce[:].rearrange("(t q) d -> q t d", q=P)
        y2fp = [feat.tile([P, GPP[p], D_HID], f16, name=f"y2f_{p}") for p in range(3)]
        h1s_gath = [
            dram.tile([GPP[p] * P, D_HID], f16, name=f"h1s_gath{p}", addr_space="Shared")
            for p in range(3)
        ]

        for ci, (c0, cw) in enumerate(CH):
            cs = slice(c0, c0 + cw)
            big_chunk(
                ci,
                [(lambda j: xs1p[_pos(j)[0]][:, _pos(j)[1], :], t1s[:, cs])],
            )
            for bo in range(2):
                bs = slice(bo * P, (bo + 1) * P)
                po = pout.tile([P, 512], f32, name="po")
                nc.tensor.matmul(po[:, :cw], w01h[:, bs], xT16[:, cs], start=True, stop=False)
                nc.tensor.matmul(po[:, :cw], w11h[:, bs], t1s[:, cs], start=False, stop=False)
                nc.tensor.matmul(po[:, :cw], b1h[:, bs], onesrow[:, cs], start=False, stop=True)
                nc.vector.tensor_relu(h1T[:, bo, cs], po[:, :cw])
            for b in range(2):
                nc.vector.tensor_tensor(h1sT[:, b, cs], h1T[:, b, cs], pdis_bc[:, cs], MUL)
            emit_gather(
                ci,
                lambda b, t: h1sT[:, b, t * P : (t + 1) * P],
                2,
                D_HID,
                h1s_bounce,
                h1s_bounce_v,
                h1s_gath[ci],
                y2fp[ci][:],
            )

        # ================= Layer 2 =================
        t2s = feat.tile([P, 2, MLOC], f16)
        h2T = feat.tile([P, 2, MLOC], f16)
        h2sT = feat.tile([P, 2, MLOC], f16)
        y3T = feat.tile([P, MLOC], f16)
        y3_bounce = dram.tile([MLOC, D_OUT], f16, name="y3_bounce")
        y3_bounce_v = y3_bounce[:].rearrange("(t q) d -> q t d", q=P)
        y3fp = [feat.tile([P, GPP[p], D_OUT], f16, name=f"y3f_{p}", tag=f"xy3_{p}") for p in range(3)]
        y3_gath = [
            dram.tile([GPP[p] * P, D_OUT], f16, name=f"y3_gath{p}", addr_space="Shared")
            for p in range(3)
        ]

        for ci, (c0, cw) in enumerate(CH):
            cs = slice(c0, c0 + cw)
            big_chunk(
                ci,
                [
                    (lambda j, b=b: y2fp[_pos(j)[0]][:, _pos(j)[1], b * P : (b + 1) * P],
                     t2s[:, b, cs])
                    for b in range(2)
                ],
            )
            for bo in range(2):
                bs = slice(bo * P, (bo + 1) * P)
                po = pout.tile([P, 512], f32, name="po")
                nc.tensor.matmul(po[:, :cw], w02h[:, 0, bs], h1T[:, 0, cs], start=True, stop=False)
                nc.tensor.matmul(po[:, :cw], w02h[:, 1, bs], h1T[:, 1, cs], start=False, stop=False)
                nc.tensor.matmul(po[:, :cw], w12h[:, 0, bs], t2s[:, 0, cs], start=False, stop=False)
                nc.tensor.matmul(po[:, :cw], w12h[:, 1, bs], t2s[:, 1, cs], start=False, stop=False)
                nc.tensor.matmul(po[:, :cw], b2h[:, bs], onesrow[:, cs], start=False, stop=True)
                nc.vector.tensor_relu(h2T[:, bo, cs], po[:, :cw])
            for b in range(2):
                nc.vector.tensor_tensor(h2sT[:, b, cs], h2T[:, b, cs], pdis_bc[:, cs], MUL)
            # Y3 = (dis .* h2) @ W13, feature-major
            py = pout.tile([P, 512], f32, name="po")
            nc.tensor.matmul(py[:, :cw], w13h[:, 0, :], h2sT[:, 0, cs], start=True, stop=False)
            nc.tensor.matmul(py[:, :cw], w13h[:, 1, :], h2sT[:, 1, cs], start=False, stop=True)
            nc.vector.tensor_copy(y3T[:, cs], py[:, :cw])
            emit_gather(
                ci,
                lambda b, t: y3T[:, t * P : (t + 1) * P],
                1,
                D_OUT,
                y3_bounce,
                y3_bounce_v,
                y3_gath[ci],
                y3fp[ci][:],
            )

        # ================= Layer 3 =================
        t3s = feat.tile([P, MLOC], f16)
        outT = feat.tile([P, MLOC], f32)
        for ci, (c0, cw) in enumerate(CH):
            cs = slice(c0, c0 + cw)
            big_chunk(
                ci,
                [(lambda j: y3fp[_pos(j)[0]][:, _pos(j)[1], :], t3s[:, cs])],
            )
            po = pout.tile([P, 512], f32, name="po")
            nc.tensor.matmul(po[:, :cw], w03h[:, 0, :], h2T[:, 0, cs], start=True, stop=False)
            nc.tensor.matmul(po[:, :cw], w03h[:, 1, :], h2T[:, 1, cs], start=False, stop=False)
            nc.tensor.matmul(po[:, :cw], b3h[:], onesrow[:, cs], start=False, stop=False)
            # += T3s (identity-matmul add of the scaled Chebyshev term)
            nc.tensor.matmul(po[:, :cw], id16[:], t3s[:, cs], start=False, stop=True)
            nc.vector.tensor_copy(outT[:, cs], po[:, :cw])
            nc.scalar.dma_start(out_d[:, cs], outT[:, cs])

    nc.compile()
    return nc


def _prep_inputs(x, edge_index, W01, W11, b1, W02, W12, b2, W03, W13, b3):
    f8 = ml_dtypes.float8_e4m3
    x = np.asarray(x, np.float32)
    ei = np.asarray(edge_index)
    src = ei[0].astype(np.int64)
    dst = ei[1].astype(np.int64)

    deg = np.bincount(src, minlength=NP).astype(np.float32)  # out-degree counts
    x_pad = np.zeros((NP, D_IN), np.float32)
    x_pad[:N] = x

    sig = np.asarray(SIGMA)
    x_nm = np.ascontiguousarray(x_pad.reshape(KT, P, D_IN)[sig]).astype(np.float16)
    deg_cols = np.ascontiguousarray(deg.reshape(KT, P)[sig].T)

    common = {
        "x_nm": x_nm,
        "deg_cols": deg_cols,
        "w01": np.ascontiguousarray(np.asarray(W01, np.float32)),
        "w11": np.ascontiguousarray(np.asarray(W11, np.float32)),
        "w02": np.ascontiguousarray(np.asarray(W02, np.float32).reshape(2, P, D_HID)),
        "w12": np.ascontiguousarray(np.asarray(W12, np.float32).reshape(2, P, D_HID)),
        "w03": np.ascontiguousarray(np.asarray(W03, np.float32).reshape(2, P, D_OUT)),
        "w13": np.ascontiguousarray(np.asarray(W13, np.float32).reshape(2, P, D_OUT)),
        "b1r": np.asarray(b1, np.float32).reshape(1, D_HID),
        "b2r": np.asarray(b2, np.float32).reshape(1, D_HID),
        "b3r": np.asarray(b3, np.float32).reshape(1, D_OUT),
    }

    in_maps = []
    for c in range(NCORES):
        lo, hi = c * MLOC, (c + 1) * MLOC
        sel = (dst >= lo) & (dst < hi)
        idx = src[sel] * MLOC + (dst[sel] - lo)
        cnt = np.bincount(idx, minlength=NP * MLOC).astype(np.float32)
        adjT = np.ascontiguousarray(cnt.reshape(KT, P, MLOC)[sig]).astype(f8)
        m = dict(common)
        m["adjT"] = adjT
        m["xT_loc"] = np.ascontiguousarray(x_pad[lo:hi].T)
        m["deg_row"] = np.ascontiguousarray(deg[lo:hi].reshape(1, MLOC))
        in_maps.append(m)
    return in_maps


def kernel(x, edge_index, edge_type, W01, W11, b1, W02, W12, b2, W03, W13, b3):
    global LAST_RESULTS
    from concourse.bass_utils import run_bass_kernel_spmd

    if "nc" not in _CACHE:
        _CACHE["nc"] = _build_nc()
    nc = _CACHE["nc"]

    in_maps = _prep_inputs(x, edge_index, W01, W11, b1, W02, W12, b2, W03, W13, b3)
    res = run_bass_kernel_spmd(
        nc,
        in_maps,
        list(range(NCORES)),
        trace=bool(os.environ.get("BASS_TRACE")),
    )
    LAST_RESULTS = res
    shards = [res.results[c]["outT"].astype(np.float32).T for c in range(NCORES)]
    out = np.concatenate(shards, axis=0)[:N]
    return np.ascontiguousarray(out)


if __name__ == "__main__":
    _build_nc()
    print("build ok")
